# revision 7
# baseline (speedup 1.0000x reference)
"""Trainium2 Bass kernel for a cross-attention block.

reference semantics (jax):
    q = x @ Wq + bq                      # (b, hw, c)
    k = p @ Wk + bk                      # (b, 77, c)
    v = p @ Wv + bv                      # (b, 77, c)
    scores = einsum("bqhd,bkhd->bhqk", q, k) / sqrt(hd)
    attn = softmax(scores, -1)
    out = einsum("bhqk,bkhd->bqhd", attn, v) @ Ww + bw

Sharding: data-parallel over batch (16 batches / 8 cores = 2 per core),
no collectives.  Inside each core everything is computed in a
"features-on-partitions" (transposed) layout so that the contraction
dim of every matmul lands on SBUF partitions:

  X^T (via PE transpose)  ->  Q^T = Wq^T @ X^T
  scores^T[77, hw] = K^T_h.T @ Q^T_h            (per head, 2-head row packing)
  exp on ScalarE (scale=1/8 folded in, no max subtraction needed --
  |scores/8| < ~3 for this problem family)
  [num; den] = [V_h | 1]^T @ exp^T              (ones-augmented V matmul)
  attn_out^T = num * (1/den)                    (gpsimd partition_broadcast)
  out[hw,c]  = attn_out^T.T @ Ww  + bw          (natural layout -> contiguous store)

All matmuls run as float32r (fp32 bits, PE fast mode: 1 cycle/row when
the moving dim >= 256 vs 4 cycles/row for plain fp32).
"""

import numpy as np
from contextlib import ExitStack

import concourse.bass as bass
import concourse.tile as tile
from concourse import bacc, mybir
from concourse.bass_utils import run_bass_kernel_spmd
from concourse.masks import make_identity

N_CORES = 8
B_FULL, HW, C = 16, 4096, 1024
NH, D, CTX, NE = 16, 64, 77, 512
B = B_FULL // N_CORES          # batches per core
P = 128
KC = C // P                    # 8 c-chunks of 128
KN = NE // P                   # 4 n_embd chunks of 128
F = 256                        # hw elements per chunk
FSUB = F // P                  # 128-row subchunks per chunk

F32 = mybir.dt.float32
F32R = mybir.dt.float32r


def _r(ap):
    """Tag an fp32 AP as float32r for the PE fast path (same bits)."""
    return ap.bitcast(F32R)


def _bcast_dram(ap, parts, free):
    """DRAM 1-D tensor broadcast across `parts` partitions (step-0 AP)."""
    return bass.AP(tensor=ap.tensor, offset=ap.offset, ap=[[0, parts], [1, free]])


def _body(ctx: ExitStack, tc: tile.TileContext, io: dict, hw: int = HW):
    nc = tc.nc
    nchunk = hw // F

    x_ap, p_ap, out_ap = io["x"], io["p"], io["out"]
    wq_ap, bq_ap = io["Wq"], io["bq"]
    wk_ap, bk_ap = io["Wk"], io["bk"]
    wv_ap, bv_ap = io["Wv"], io["bv"]
    ww_ap, bw_ap = io["Ww"], io["bw"]

    # ---------------- pools ----------------
    # NOTE: pool address space is claimed in open order, so phase-B pools are
    # opened only after the phase-A scratch scope (wkv/ppool) closes.
    consts = ctx.enter_context(tc.tile_pool(name="consts", bufs=1))
    wpool = ctx.enter_context(tc.tile_pool(name="wpool", bufs=1))
    kvout = ctx.enter_context(tc.tile_pool(name="kvout", bufs=1))
    # PSUM: tags "tp"(1) + "qk"(2) + "at"(3) + "fin"(2) = 8 banks
    ps_tp = ctx.enter_context(tc.tile_pool(name="ps_tp", bufs=1, space="PSUM"))
    ps_qk = ctx.enter_context(tc.tile_pool(name="ps_qk", bufs=2, space="PSUM"))
    ps_at = ctx.enter_context(tc.tile_pool(name="ps_at", bufs=3, space="PSUM"))
    ps_fin = ctx.enter_context(tc.tile_pool(name="ps_fin", bufs=2, space="PSUM"))

    # ---------------- constants ----------------
    ident = consts.tile([P, P], F32, name="ident")
    make_identity(nc, ident[:])

    # per-cout-chunk bias columns: bq_sb[:, mc] == bq[mc*128 : (mc+1)*128]
    bq_sb = consts.tile([P, KC], F32, name="bq_sb")
    nc.sync.dma_start(out=bq_sb[:], in_=bq_ap.rearrange("(a b) -> b a", b=P))
    bk_sb = consts.tile([P, KC], F32, name="bk_sb")
    nc.sync.dma_start(out=bk_sb[:], in_=bk_ap.rearrange("(a b) -> b a", b=P))
    # free-dim biases broadcast across partitions (done once via DRAM DMA)
    bv_bc = consts.tile([CTX, C], F32, name="bv_bc")
    nc.sync.dma_start(out=bv_bc[:], in_=_bcast_dram(bv_ap, CTX, C))
    bw_bc = consts.tile([P, C], F32, name="bw_bc")
    nc.sync.dma_start(out=bw_bc[:], in_=_bcast_dram(bw_ap, P, C))

    # resident weights: Wq / Ww as 8 [128, 1024] k-slices (lhsT-ready)
    wq = []
    for k in range(KC):
        t = wpool.tile([P, C], F32R, name=f"wq{k}", tag=f"wq{k}")
        nc.sync.dma_start(out=t[:], in_=wq_ap[k * P : (k + 1) * P, :].bitcast(F32R))
        wq.append(t)
    ww = []
    for k in range(KC):
        t = wpool.tile([P, C], F32R, name=f"ww{k}", tag=f"ww{k}")
        nc.sync.dma_start(out=t[:], in_=ww_ap[k * P : (k + 1) * P, :].bitcast(F32R))
        ww.append(t)

    # K^T tiles [128, 77] per (batch, c-chunk); V augmented [77, NH, D+1]
    kT = [
        [kvout.tile([P, CTX], F32R, name=f"kT{b}_{m}", tag=f"kT{b}_{m}") for m in range(KC)]
        for b in range(B)
    ]
    v_aug = [
        kvout.tile([CTX, NH, D + 1], F32R, name=f"vaug{b}", tag=f"vaug{b}")
        for b in range(B)
    ]

    # ---------------- phase A: K/V projections (tiny) ----------------
    with ExitStack() as kvctx:
        wkv = kvctx.enter_context(tc.tile_pool(name="wkv", bufs=1))
        ppool = kvctx.enter_context(tc.tile_pool(name="ppool", bufs=2))
        wk = []
        wv = []
        for k in range(KN):
            t = wkv.tile([P, C], F32R, name=f"wk{k}", tag=f"wk{k}")
            nc.sync.dma_start(out=t[:], in_=wk_ap[k * P : (k + 1) * P, :].bitcast(F32R))
            wk.append(t)
            t = wkv.tile([P, C], F32R, name=f"wv{k}", tag=f"wv{k}")
            nc.sync.dma_start(out=t[:], in_=wv_ap[k * P : (k + 1) * P, :].bitcast(F32R))
            wv.append(t)

        for b in range(B):
            # p[b] natural [77, 512], then PE-transpose into pT [4][128, 77]
            pnat = ppool.tile([CTX, NE], F32, name="pnat", tag="pnat", bufs=2)
            nc.sync.dma_start(out=pnat[:], in_=p_ap[b])
            pT = []
            for k in range(KN):
                ps = ps_tp.tile([P, CTX], F32, name="ps_pT", tag="tp")
                nc.tensor.transpose(ps[:], pnat[:, k * P : (k + 1) * P], ident[:CTX, :CTX])
                t = ppool.tile([P, CTX], F32R, name=f"pT{k}", tag=f"pT{k}", bufs=2)
                nc.vector.tensor_copy(out=t[:], in_=ps[:])
                pT.append(t)

            # K^T[mc] = sum_k Wk[k,mc-slice].T @ pT[k]  (+ bk)
            for mc in range(KC):
                ps = ps_qk.tile([P, CTX], F32, name="ps_kT", tag="qk")
                for k in range(KN):
                    # N=77 is illegal for the fp32r fast path; plain fp32 here
                    # (tiny: 32 matmuls per batch).
                    nc.tensor.matmul(
                        ps[:],
                        wk[k][:, mc * P : (mc + 1) * P].bitcast(F32),
                        pT[k][:].bitcast(F32),
                        start=(k == 0),
                        stop=(k == KN - 1),
                    )
                nc.vector.tensor_add(
                    kT[b][mc][:], ps[:], bk_sb[:, mc : mc + 1].to_broadcast([P, CTX])
                )

            # V natural [77, c]: lhsT = pT[k] (K=128, M=77), rhs = Wv slice
            for nb in range(C // 512):
                ps = ps_at.tile([CTX, 512], F32, name="ps_v", tag="at")
                for k in range(KN):
                    nc.tensor.matmul(
                        ps[:],
                        pT[k][:],
                        wv[k][:, nb * 512 : (nb + 1) * 512],
                        start=(k == 0),
                        stop=(k == KN - 1),
                    )
                nc.vector.tensor_add(
                    v_aug[b][:, nb * 8 : (nb + 1) * 8, 0:D],
                    ps[:].rearrange("p (h d) -> p h d", d=D),
                    bv_bc[:, nb * 512 : (nb + 1) * 512].rearrange(
                        "p (h d) -> p h d", d=D
                    ),
                )
            # ones column for the fused softmax denominator
            nc.vector.memset(v_aug[b][:, :, D : D + 1].bitcast(F32), 1.0)

    # ---------------- phase B: main loop ----------------
    xpool = ctx.enter_context(tc.tile_pool(name="xpool", bufs=1))
    qpool = ctx.enter_context(tc.tile_pool(name="qpool", bufs=1))
    apool = ctx.enter_context(tc.tile_pool(name="apool", bufs=1))
    epool = ctx.enter_context(tc.tile_pool(name="epool", bufs=4))
    opool = ctx.enter_context(tc.tile_pool(name="opool", bufs=4))
    spool = ctx.enter_context(tc.tile_pool(name="spool", bufs=4))
    for b in range(B):
        for j in range(nchunk):
            r0 = j * F
            # x chunk natural [2][128, 1024]
            xn = []
            for r in range(FSUB):
                t = xpool.tile([P, C], F32, name="xn", tag="xn", bufs=4)
                nc.sync.dma_start(
                    out=t[:], in_=x_ap[b, r0 + r * P : r0 + (r + 1) * P, :]
                )
                xn.append(t)
            # PE-transpose -> xT[kc] [128(c), 256(hw)]
            xT = []
            for kc in range(KC):
                ps = ps_tp.tile([P, F], F32, name="ps_xT", tag="tp")
                for r in range(FSUB):
                    nc.tensor.transpose(
                        ps[:, r * P : (r + 1) * P],
                        xn[r][:, kc * P : (kc + 1) * P],
                        ident[:],
                    )
                t = xpool.tile([P, F], F32R, name="xT", tag="xT", bufs=16)
                nc.vector.tensor_copy(out=t[:], in_=ps[:])
                xT.append(t)

            # Q^T[mc] = sum_kc Wq[kc, mc-slice].T @ xT[kc]  (+ bq)
            qT = []
            for mc in range(KC):
                ps = ps_qk.tile([P, F], F32, name="ps_qT", tag="qk")
                for kc in range(KC):
                    nc.tensor.matmul(
                        ps[:],
                        wq[kc][:, mc * P : (mc + 1) * P],
                        xT[kc][:],
                        start=(kc == 0),
                        stop=(kc == KC - 1),
                    )
                t = qpool.tile([P, F], F32R, name="qT", tag="qT", bufs=16)
                nc.vector.tensor_add(
                    t[:], ps[:], bq_sb[:, mc : mc + 1].to_broadcast([P, F])
                )
                qT.append(t)

            # attention per head; attn-out^T accumulates into aT[kc][128, 256]
            aT = [
                apool.tile([P, F], F32R, name="aT", tag="aT", bufs=16)
                for _ in range(KC)
            ]
            for h in range(NH):
                mc, half = h // 2, (h % 2) * D
                # scores^T [77, F] = kT_h.T @ qT_h   (K = 64, row-packed pairs)
                ps_s = ps_at.tile([CTX, F], F32, name="ps_s", tag="at")
                nc.tensor.matmul(
                    ps_s[:],
                    kT[b][mc][half : half + D, :],
                    qT[mc][half : half + D, :],
                    start=True,
                    stop=True,
                    tile_position=(half, 0),
                )
                # exp(scores / 8) on ScalarE straight out of PSUM
                ex = epool.tile([CTX, F], F32R, name="ex", tag="ex")
                nc.scalar.activation(
                    ex[:], ps_s[:], mybir.ActivationFunctionType.Exp, scale=0.125
                )
                # [numerator; denominator] in one matmul via ones-augmented V
                ps_o = ps_at.tile([D + 1, F], F32, name="ps_o", tag="at")
                nc.tensor.matmul(
                    ps_o[:], v_aug[b][:, h, :], ex[:], start=True, stop=True
                )
                inv = spool.tile([1, F], F32, name="inv", tag="inv")
                nc.vector.reciprocal(out=inv[:], in_=ps_o[D : D + 1, :])
                bc = spool.tile([D, F], F32, name="bc", tag="bc")
                nc.gpsimd.partition_broadcast(bc[:], inv[:])
                nc.vector.tensor_mul(aT[mc][half : half + D, :], ps_o[0:D, :], bc[:])

            # final projection, natural orientation: out[hw128, c]
            for fs in range(FSUB):
                osb = opool.tile([P, C], F32, name="osb", tag="osb")
                for nb in range(C // 512):
                    ps = ps_fin.tile([P, 512], F32, name="ps_f", tag="fin")
                    for kc in range(KC):
                        nc.tensor.matmul(
                            ps[:],
                            aT[kc][:, fs * P : (fs + 1) * P],
                            ww[kc][:, nb * 512 : (nb + 1) * 512],
                            start=(kc == 0),
                            stop=(kc == KC - 1),
                        )
                    nc.vector.tensor_add(
                        osb[:, nb * 512 : (nb + 1) * 512],
                        ps[:],
                        bw_bc[:, nb * 512 : (nb + 1) * 512],
                    )
                nc.sync.dma_start(
                    out=out_ap[b, r0 + fs * P : r0 + (fs + 1) * P, :], in_=osb[:]
                )


def build_program(hw: int = HW):
    """Build + compile the per-core Bass program (SPMD, identical per core)."""
    nc = bacc.Bacc(
        "TRN2", target_bir_lowering=False, debug=False, num_devices=N_CORES
    )
    io = {}
    io["x"] = nc.dram_tensor("x", [B, hw, C], F32, kind="ExternalInput").ap()
    io["p"] = nc.dram_tensor("p", [B, CTX, NE], F32, kind="ExternalInput").ap()
    for name, shape in [
        ("Wq", [C, C]),
        ("bq", [C]),
        ("Wk", [NE, C]),
        ("bk", [C]),
        ("Wv", [NE, C]),
        ("bv", [C]),
        ("Ww", [C, C]),
        ("bw", [C]),
    ]:
        io[name] = nc.dram_tensor(name, shape, F32, kind="ExternalInput").ap()
    io["out"] = nc.dram_tensor("out", [B, hw, C], F32, kind="ExternalOutput").ap()

    with tile.TileContext(nc) as tc:
        with ExitStack() as ctx:
            _body(ctx, tc, io, hw=hw)
    nc.compile()
    return nc


_PROGRAM = None


def run_sharded(inputs: dict, trace: bool = False, **trace_kwargs):
    """Shard inputs over the 8 cores, run, gather. Returns (out, results)."""
    global _PROGRAM
    if _PROGRAM is None:
        _PROGRAM = build_program()
    nc = _PROGRAM

    full = {
        k: np.ascontiguousarray(v, dtype=np.float32)
        for k, v in inputs.items()
    }
    in_maps = []
    for i in range(N_CORES):
        m = dict(full)
        m["x"] = full["x"][i * B : (i + 1) * B]
        m["p"] = full["p"][i * B : (i + 1) * B]
        in_maps.append(m)

    res = run_bass_kernel_spmd(
        nc, in_maps, list(range(N_CORES)), trace=trace, **trace_kwargs
    )
    out = np.concatenate([res.results[i]["out"] for i in range(N_CORES)], axis=0)
    return out, res


def kernel(x, p, Wq, bq, Wk, bk, Wv, bv, Ww, bw):
    out, _ = run_sharded(
        dict(x=x, p=p, Wq=Wq, bq=bq, Wk=Wk, bk=bk, Wv=Wv, bv=bv, Ww=Ww, bw=bw)
    )
    return out
